# revision 15
# baseline (speedup 1.0000x reference)
"""Trainium2 Bass kernel for nn_ManyBodyPadAttn.

Computation (see reference):
  Q  = feat1 @ Wq.T + bq            [B,I,J,C]   (scaled by HEAD_DIM^-0.5 after)
  KV = feat2 @ Wkv.T + bkv          [B,J,K,2C]
  EG = feat2 @ Weg.T + beg          [B,J,K,2H]
  H  = einsum('bijdh,bjkdh->bijkh', Q, K) + E
  A  = softmax_k(H) * sigmoid(G)
  Va = einsum('bijkh,bjkdh->bijdh', A, V)  -> [B,I,J,C] -> layernorm(C)

Sharding: J axis across the 8 cores (16 j's per core). Every tensor and all
FLOPs shard cleanly by J (output carries J; K/V/E/G are per-(j,k); Q per
(i,j)) -- no replicated compute and no collectives.

Per-core kernel strategy:
  - host pre-transposes activations to [b, c, j*128+row] bf16 so the
    contraction dim (c) is on partitions with zero on-chip transposes
  - head channels are permuted on the host (c' = h*32+d) so each head is a
    contiguous 32-partition block; QK^T runs per head via PE tiling
    (base-partition offsets), contracting d=32
  - softmax is folded: S^T = K_h^T.T @ Q_h^T per (b,j,h), P = exp(S^T),
    V'' = V' * (exp(E)*sigmoid(G)) with an extra column of exp(E); then
    Va_aug = P^T.T @ V''_aug gives both the numerator and the softmax
    denominator from one matmul; divide + un-permute + layernorm follow
  - rsqrt for LN via fast-inverse-sqrt bit trick + 3 Newton steps (avoids
    ACT table-set switches; only the exp/tanh table set is ever loaded)
"""

import os
import sys

sys.path.insert(0, "/opt/trn_rl_repo")

import numpy as np
import ml_dtypes

B, N, C, H, D = 2, 128, 256, 8, 32
NCORES = 8
JP = N // NCORES          # j's per core
JPN = JP * N              # free extent of (j, row) blocks

_BUILD_CACHE = {}


def _build(flags):
    """Build + bacc-compile the per-core Bass program. flags is a tuple
    (has_bq, has_bk, has_bveg, has_gb)."""
    from concourse import bass, bacc, mybir, tile
    from concourse.alu_op_type import AluOpType as OP

    has_bq, has_bk, has_bveg, has_gb = flags
    AF = mybir.ActivationFunctionType
    F32 = mybir.dt.float32
    BF16 = mybir.dt.bfloat16
    I32 = mybir.dt.int32

    nc = bacc.Bacc("TRN2", target_bir_lowering=False, debug=False, num_devices=NCORES)

    f1t = nc.dram_tensor("f1t", [B, C, JPN], BF16, kind="ExternalInput").ap()
    f2t = nc.dram_tensor("f2t", [B, C, JPN], BF16, kind="ExternalInput").ap()
    wqt = nc.dram_tensor("wqt", [C, C], BF16, kind="ExternalInput").ap()
    wkt = nc.dram_tensor("wkt", [C, C], BF16, kind="ExternalInput").ap()
    wvt = nc.dram_tensor("wvt", [C, C], BF16, kind="ExternalInput").ap()
    wegt = nc.dram_tensor("wegt", [C, 2 * H], BF16, kind="ExternalInput").ap()
    if has_bq:
        bq_d = nc.dram_tensor("bq_p", [C], F32, kind="ExternalInput").ap()
    if has_bk:
        bk_d = nc.dram_tensor("bk_p", [C], F32, kind="ExternalInput").ap()
    if has_bveg:
        bveg_d = nc.dram_tensor("bveg_p", [C + 2 * H], BF16, kind="ExternalInput").ap()
    if has_gb:
        gamma_d = nc.dram_tensor("gamma_p", [C], F32, kind="ExternalInput").ap()
        beta_d = nc.dram_tensor("beta_p", [C], F32, kind="ExternalInput").ap()
    out_t = nc.dram_tensor("out", [B, N, JP, C], F32, kind="ExternalOutput").ap()

    from contextlib import ExitStack

    with tile.TileContext(nc) as tc, ExitStack() as ctx:
        singles = ctx.enter_context(tc.tile_pool(name="singles", bufs=1))

        f1t_sb = singles.tile([128, B, 2, JPN], BF16)
        f2t_sb = singles.tile([128, B, 2, JPN], BF16)
        qt_sb = singles.tile([128, B, 2, JPN], BF16)
        kt_sb = singles.tile([128, B, 2, JPN], BF16)
        qt_x = singles.tile([128, B, JPN], BF16)
        kt_x = singles.tile([128, B, JPN], BF16)
        qt_x2 = singles.tile([128, B, JPN], BF16)
        kt_x2 = singles.tile([128, B, JPN], BF16)
        wqt_sb = singles.tile([128, 2, C], BF16)
        wkt_sb = singles.tile([128, 2, C], BF16)
        wvt_sb = singles.tile([128, 2, C], BF16)
        wegt_sb = singles.tile([128, 2, 2 * H], BF16)
        magic_sb = singles.tile([128, 1], I32)
        nc.vector.memset(magic_sb[:], 0x5F3759DF)

        # weights first in the DMA queue (everything needs them)
        nc.sync.dma_start(out=wqt_sb[:], in_=wqt.rearrange("(cc p) n -> p cc n", p=128))
        nc.sync.dma_start(out=wkt_sb[:], in_=wkt.rearrange("(cc p) n -> p cc n", p=128))
        nc.sync.dma_start(out=wvt_sb[:], in_=wvt.rearrange("(cc p) n -> p cc n", p=128))
        nc.sync.dma_start(out=wegt_sb[:], in_=wegt.rearrange("(cc p) n -> p cc n", p=128))
        if has_bq:
            bq_sb = singles.tile([128, 2], F32)
            nc.sync.dma_start(out=bq_sb[:], in_=bq_d.rearrange("(m p) -> p m", p=128))
        if has_bk:
            bk_sb = singles.tile([128, 2], F32)
            nc.sync.dma_start(out=bk_sb[:], in_=bk_d.rearrange("(m p) -> p m", p=128))
        if has_bveg:
            ones_sb = singles.tile([1, 128], BF16)
            nc.vector.memset(ones_sb[:], 1.0)
            bveg_sb = singles.tile([1, C + 2 * H], BF16)
            nc.sync.dma_start(out=bveg_sb[:], in_=bveg_d.rearrange("(one n) -> one n", one=1))
        if has_gb:
            gamma_sb = singles.tile([128, C], F32)
            beta_sb = singles.tile([128, C], F32)
            nc.sync.dma_start(out=gamma_sb[:], in_=bass.AP(
                tensor=gamma_d.tensor, offset=gamma_d.offset, ap=[[0, 128], [1, C]]))
            nc.sync.dma_start(out=beta_sb[:], in_=bass.AP(
                tensor=beta_d.tensor, offset=beta_d.offset, ap=[[0, 128], [1, C]]))

        # feature loads, ordered so the earliest-needed chunks arrive first:
        # per b: f2t (EG prepass + K/V) then f1t (Q), in g-column chunks with
        # both contraction halves back to back
        def emit_feat_loads(b, nq):
            qw = 2048 // nq
            for ft, sb in ((f2t, f2t_sb), (f1t, f1t_sb)):
                for q in range(nq):
                    for cc in range(2):
                        sl = slice(q * qw, (q + 1) * qw)
                        nc.sync.dma_start(out=sb[:, b, cc, sl],
                                          in_=ft[b, cc * 128:(cc + 1) * 128, sl])

        emit_feat_loads(0, 4)

        # vp: V' projection only. px: EG prepass + interleaved b=1 stage-1
        # units (kept separate so waiting-on-b1-data tiles can never block V')
        vp_pool = ctx.enter_context(tc.tile_pool(name="vp", bufs=1, space="PSUM"))
        px_pool = ctx.enter_context(tc.tile_pool(name="px", bufs=1, space="PSUM"))
        egout = ctx.enter_context(tc.tile_pool(name="egout", bufs=2))

        def eg_prepass(b):
            egps = px_pool.tile([128, JP * 2 * H], F32, name=f"egps{b}", tag="px")
            for j in range(JP):
                for cc in range(2):
                    nc.tensor.matmul(
                        out=egps[:, j * 16:(j + 1) * 16],
                        lhsT=f2t_sb[:, b, cc, j * 128:(j + 1) * 128],
                        rhs=wegt_sb[:, cc, :],
                        start=(cc == 0), stop=(cc == 1 and not has_bveg))
                if has_bveg:
                    nc.tensor.matmul(
                        out=egps[:, j * 16:(j + 1) * 16],
                        lhsT=ones_sb[:], rhs=bveg_sb[:, C:C + 16],
                        start=False, stop=True)
            eg3 = egps.rearrange("p (j c) -> p j c", j=JP)
            w_t = egout.tile([128, JP, H], F32, name=f"w{b}", tag="w")
            t_t = egout.tile([128, JP, H], F32, name=f"t{b}", tag="t")
            nc.scalar.activation(out=w_t[:], in_=eg3[:, :, 0:H], func=AF.Exp)
            nc.scalar.activation(out=t_t[:], in_=eg3[:, :, H:2 * H], func=AF.Tanh, scale=0.5)
            sig_t = egout.tile([128, JP, H], F32, name=f"sig{b}", tag="sig")
            nc.gpsimd.tensor_scalar(out=sig_t[:], in0=t_t[:], scalar1=0.5, scalar2=0.5,
                                    op0=OP.mult, op1=OP.add)
            ws_t = egout.tile([128, JP, H], F32, name=f"ws{b}", tag="ws")
            nc.gpsimd.tensor_tensor(out=ws_t[:], in0=w_t[:], in1=sig_t[:], op=OP.mult)
            return w_t, ws_t

        def stage1_unit(pool, tag, b, g, which, m):
            """Project one [128, 512] column group of Q^T or K^T."""
            src_sb, w_sb, dst_sb = ((f2t_sb, wkt_sb, kt_sb) if which == "k"
                                    else (f1t_sb, wqt_sb, qt_sb))
            tl = pool.tile([128, 512], F32, name=f"pj_{which}{b}{g}{m}", tag=tag)
            for cc in range(2):
                nc.tensor.matmul(out=tl[:],
                                 lhsT=w_sb[:, cc, m * 128:(m + 1) * 128],
                                 rhs=src_sb[:, b, cc, g * 512:(g + 1) * 512],
                                 start=(cc == 0), stop=(cc == 1))
            dst = dst_sb[:, b, m, g * 512:(g + 1) * 512]
            if which == "q":
                if has_bq:
                    nc.scalar.activation(out=dst, in_=tl[:], func=AF.Identity,
                                         bias=bq_sb[:, m:m + 1], scale=1.0)
                else:
                    nc.scalar.activation(out=dst, in_=tl[:], func=AF.Copy)
            else:
                if has_bk:
                    nc.scalar.activation(out=dst, in_=tl[:], func=AF.Identity,
                                         bias=bk_sb[:, m:m + 1], scale=1.0)
                else:
                    nc.scalar.activation(out=dst, in_=tl[:], func=AF.Copy)

        def emit_xtiles(b):
            # relocate rows 96:128 (heads 3/7) and 64:96 (heads 2/6) so only
            # PE row-tiles 0 and 32 are ever used (2 PSUM banks for S)
            for m in range(2):
                nc.sync.dma_start(out=qt_x[m * 32:(m + 1) * 32, b, :], in_=qt_sb[96:128, b, m, :])
                nc.sync.dma_start(out=kt_x[m * 32:(m + 1) * 32, b, :], in_=kt_sb[96:128, b, m, :])
                nc.sync.dma_start(out=qt_x2[m * 32:(m + 1) * 32, b, :], in_=qt_sb[64:96, b, m, :])
                nc.sync.dma_start(out=kt_x2[m * 32:(m + 1) * 32, b, :], in_=kt_sb[64:96, b, m, :])

        eg_res = {0: eg_prepass(0)}
        with tc.tile_pool(name="pj0", bufs=6, space="PSUM") as pj0:
            for g in range(4):
                for which in ("k", "q"):
                    for m in range(2):
                        stage1_unit(pj0, "pj", 0, g, which, m)
        emit_xtiles(0)
        emit_feat_loads(1, 1)

        # ---- phase 2 pools ----------------------------------------------
        s_pool = ctx.enter_context(tc.tile_pool(name="sp", bufs=2, space="PSUM"))
        va_pool = ctx.enter_context(tc.tile_pool(name="vap", bufs=2, space="PSUM"))
        pj1 = px_pool
        pt_pool = ctx.enter_context(tc.tile_pool(name="ptp", bufs=2))
        vaug_pool = ctx.enter_context(tc.tile_pool(name="vaugp", bufs=2))
        van_pool = ctx.enter_context(tc.tile_pool(name="vanp", bufs=10))
        sq_pool = ctx.enter_context(tc.tile_pool(name="sqp", bufs=2))
        out_pool = ctx.enter_context(tc.tile_pool(name="outp", bufs=4))
        rd_pool = ctx.enter_context(tc.tile_pool(name="rdp", bufs=3))
        st_pool = ctx.enter_context(tc.tile_pool(name="stp", bufs=2))

        eg_res[1] = eg_prepass(1)
        stage1_b1 = [(g, which, m) for g in range(4) for which in ("k", "q")
                     for m in range(2)]
        s1_idx = 0

        GS = 4  # pairs per LN-stats group
        for b in range(B):
            w_t, ws_t = eg_res[b]
            msum = sqsum = None
            vans = []
            for j in range(JP):
                if b == 0 and s1_idx < len(stage1_b1):
                    g, which, m = stage1_b1[s1_idx]
                    stage1_unit(pj1, "px", 1, g, which, m)
                    s1_idx += 1
                    if s1_idx == len(stage1_b1):
                        emit_xtiles(1)
                if j % GS == 0:
                    msum = st_pool.tile([128, GS], F32, name=f"msum{b}_{j}", tag="msum")
                    sqsum = st_pool.tile([128, GS], F32, name=f"sqsum{b}_{j}", tag="sqsum")

                # V' projection: [k, c'] for this (b, j)
                vp = vp_pool.tile([128, C], F32, name=f"vp{b}_{j}", tag="vp")
                for cc in range(2):
                    nc.tensor.matmul(
                        out=vp[:],
                        lhsT=f2t_sb[:, b, cc, j * 128:(j + 1) * 128],
                        rhs=wvt_sb[:, cc, :],
                        start=(cc == 0), stop=(cc == 1 and not has_bveg))
                if has_bveg:
                    nc.tensor.matmul(out=vp[:], lhsT=ones_sb[:], rhs=bveg_sb[:, 0:C],
                                     start=False, stop=True)

                # V''_aug: per head 32 scaled V columns + 1 column of exp(E)
                vaug = vaug_pool.tile([128, H * (D + 1)], BF16, name=f"vaug{b}_{j}", tag="vaug")
                vaug3 = vaug.rearrange("p (h x) -> p h x", h=H)
                wsj = ws_t[:, j, :]
                ws_bc = bass.AP(tensor=wsj.tensor, offset=wsj.offset,
                                ap=[wsj.ap[0], [1, H], [0, D]])
                nc.vector.tensor_tensor(out=vaug3[:, :, 0:D],
                                        in0=vp.rearrange("p (h d) -> p h d", h=H),
                                        in1=ws_bc, op=OP.mult)
                nc.gpsimd.tensor_copy(out=vaug3[:, :, D:D + 1],
                                      in_=w_t[:, j, :].rearrange("p (h one) -> p h one", one=1))

                # S^T = K_h^T.T @ Q_h^T. Only PE row-tiles 0/32 are used
                # (heads 2,3,6,7 read from relocation tiles), so S fits two
                # fully-packed PSUM banks: col = rt*512 + sub*128.
                s_t = s_pool.tile([128, 1024], F32, name=f"s{b}_{j}", tag="s")
                jsl = slice(j * 128, (j + 1) * 128)
                srcs = {
                    0: (kt_sb[0:32, b, 0, jsl], qt_sb[0:32, b, 0, jsl], 0, 0),
                    4: (kt_sb[0:32, b, 1, jsl], qt_sb[0:32, b, 1, jsl], 0, 1),
                    3: (kt_x[0:32, b, jsl], qt_x[0:32, b, jsl], 0, 2),
                    2: (kt_x2[0:32, b, jsl], qt_x2[0:32, b, jsl], 0, 3),
                    1: (kt_sb[32:64, b, 0, jsl], qt_sb[32:64, b, 0, jsl], 1, 0),
                    5: (kt_sb[32:64, b, 1, jsl], qt_sb[32:64, b, 1, jsl], 1, 1),
                    7: (kt_x[32:64, b, jsl], qt_x[32:64, b, jsl], 1, 2),
                    6: (kt_x2[32:64, b, jsl], qt_x2[32:64, b, jsl], 1, 3),
                }
                for h in range(H):
                    lhs, rhs, rt, sub = srcs[h]
                    col = rt * 512 + sub * 128
                    nc.tensor.matmul(out=s_t[:, col:col + 128],
                                     lhsT=lhs, rhs=rhs, start=True, stop=True)
                pt = pt_pool.tile([128, 1024], BF16, name=f"pt{b}_{j}", tag="pt")
                nc.scalar.activation(out=pt[:], in_=s_t[:], func=AF.Exp)

                # Va_aug[i, (h, d|denom)] = sum_k P[k,i] * V''_aug[k, ...]
                va = va_pool.tile([128, H * (D + 1)], F32, name=f"va{b}_{j}", tag="va")
                va3 = va.rearrange("p (h x) -> p h x", h=H)
                for h in range(H):
                    rt, sub = srcs[h][2], srcs[h][3]
                    g2 = rt * 4 + sub
                    nc.tensor.matmul(
                        out=va3[:, h, :],
                        lhsT=pt[:, g2 * 128:(g2 + 1) * 128],
                        rhs=vaug3[:, h, :],
                        start=True, stop=True)

                # softmax denominators -> reciprocals
                rd = rd_pool.tile([128, H], F32, name=f"rd{b}_{j}", tag="rd")
                nc.vector.reciprocal(out=rd.rearrange("p (h one) -> p h one", one=1),
                                     in_=va3[:, :, D:D + 1])

                # Va_n[i, d*8+h] = Va[i,h,d] * rd[i,h]; accumulate row-sum
                van = van_pool.tile([128, C], F32, name=f"van{b}_{j}", tag="van")
                van_perm = bass.AP(tensor=van.tensor, offset=van.offset,
                                   ap=[van.ap[0], [1, H], [H, D]])
                rd_bc = bass.AP(tensor=rd.tensor, offset=rd.offset,
                                ap=[rd.ap[0], [1, H], [0, D]])
                nc.vector.scalar_tensor_tensor(out=van_perm, in0=va3[:, :, 0:D], scalar=1.0,
                                               in1=rd_bc, op0=OP.bypass, op1=OP.mult,
                                               accum_out=msum[:, j % GS:j % GS + 1])
                # sum of squares for the variance
                sq = sq_pool.tile([128, C], F32, name=f"sq{b}_{j}", tag="sq")
                nc.gpsimd.tensor_tensor(out=sq[:], in0=van[:], in1=van[:], op=OP.mult)
                nc.vector.tensor_reduce(out=sqsum[:, j % GS:j % GS + 1], in_=sq[:],
                                        axis=mybir.AxisListType.X, op=OP.add)
                vans.append(van)
                if j % GS != GS - 1:
                    continue
                g0 = j - GS + 1
                # ---- LN stats for this group of GS pairs; on GPSIMD except
                # the tail-critical last group (DVE = shorter latency) ------
                ve = nc.vector if (j == JP - 1) else nc.gpsimd
                m_t = st_pool.tile([128, GS], F32, name=f"mean{b}_{j}", tag="mean")
                ve.tensor_scalar(out=m_t[:], in0=msum[:], scalar1=1.0 / C, scalar2=0.0,
                                        op0=OP.mult, op1=OP.bypass)
                ex2 = st_pool.tile([128, GS], F32, name=f"ex2{b}_{j}", tag="ex2")
                ve.tensor_scalar(out=ex2[:], in0=sqsum[:], scalar1=1.0 / C, scalar2=1e-3,
                                        op0=OP.mult, op1=OP.add)
                mm_t = st_pool.tile([128, GS], F32, name=f"mm{b}_{j}", tag="mm")
                ve.tensor_tensor(out=mm_t[:], in0=m_t[:], in1=m_t[:], op=OP.mult)
                veps = st_pool.tile([128, GS], F32, name=f"veps{b}_{j}", tag="veps")
                ve.tensor_tensor(out=veps[:], in0=ex2[:], in1=mm_t[:], op=OP.subtract)
                u_t = st_pool.tile([128, GS], I32, name=f"u{b}_{j}", tag="u")
                nc.vector.tensor_scalar(out=u_t[:], in0=veps.bitcast(I32), scalar1=1, scalar2=0,
                                        op0=OP.logical_shift_right, op1=OP.bypass)
                y_t = st_pool.tile([128, GS], F32, name=f"y{b}_{j}", tag="y")
                magic_bc = bass.AP(tensor=magic_sb.tensor, offset=magic_sb.offset,
                                   ap=[magic_sb.ap[0], [0, GS]])
                nc.vector.scalar_tensor_tensor(out=y_t.bitcast(I32), in0=u_t[:], scalar=-1.0,
                                               in1=magic_bc, op0=OP.mult, op1=OP.add)
                tn = st_pool.tile([128, GS], F32, name=f"tn{b}_{j}", tag="tn")
                for _ in range(2):
                    ve.tensor_tensor(out=tn[:], in0=y_t[:], in1=y_t[:], op=OP.mult)
                    ve.tensor_tensor(out=tn[:], in0=tn[:], in1=veps[:], op=OP.mult)
                    ve.tensor_scalar(out=tn[:], in0=tn[:], scalar1=-0.5, scalar2=1.5,
                                            op0=OP.mult, op1=OP.add)
                    ve.tensor_tensor(out=y_t[:], in0=y_t[:], in1=tn[:], op=OP.mult)
                # ---- finalize + store the group (one DMA per group) -----
                o_t = out_pool.tile([128, GS, C], F32, name=f"o{b}_{j}", tag="o")
                for u in range(GS):
                    nc.vector.tensor_scalar(out=o_t[:, u, :], in0=vans[g0 + u][:],
                                            scalar1=m_t[:, u:u + 1],
                                            scalar2=y_t[:, u:u + 1],
                                            op0=OP.subtract, op1=OP.mult)
                    if has_gb:
                        nc.gpsimd.tensor_tensor(out=o_t[:, u, :], in0=o_t[:, u, :],
                                                in1=gamma_sb[:], op=OP.mult)
                        nc.gpsimd.tensor_tensor(out=o_t[:, u, :], in0=o_t[:, u, :],
                                                in1=beta_sb[:], op=OP.add)
                nc.sync.dma_start(out=out_t[b, :, g0:g0 + GS, :], in_=o_t[:])

    nc.compile()
    return nc


def _numpy_fallback(feat1, feat2, mask, Wq, bq, Wkv, bkv, Weg, beg, ln_gamma, ln_beta):
    f1 = feat1.astype(np.float64)
    f2 = feat2.astype(np.float64)
    Q = f1 @ Wq.T.astype(np.float64) + bq
    KV = f2 @ Wkv.T.astype(np.float64) + bkv
    K_in, V_in = np.split(KV, 2, axis=-1)
    EG = (f2 @ Weg.T.astype(np.float64) + beg)[:, None]
    E_in, G_in = np.split(EG, 2, axis=-1)

    def sh(x):
        return x.reshape(*x.shape[:3], D, H)

    Q = sh(Q) * (D ** -0.5)
    K_in = sh(K_in)
    V_in = sh(V_in)
    Hl = np.einsum("bijdh,bjkdh->bijkh", Q, K_in) + E_in
    Hl = np.where(mask[..., None], Hl, np.finfo(np.float32).min)
    Hl = Hl - Hl.max(axis=3, keepdims=True)
    Ex = np.exp(Hl)
    A = Ex / Ex.sum(axis=3, keepdims=True)
    A = A * (1.0 / (1.0 + np.exp(-G_in)))
    Va = np.einsum("bijkh,bjkdh->bijdh", A, V_in)
    Va = Va.reshape(*Va.shape[:3], C)
    m = Va.mean(-1, keepdims=True)
    v = Va.var(-1, keepdims=True)
    out = (Va - m) / np.sqrt(v + 1e-3) * ln_gamma + ln_beta
    return out.astype(np.float32)


def kernel(feat1, feat2, mask, Wq, bq, Wkv, bkv, Weg, beg, ln_gamma, ln_beta):
    feat1 = np.asarray(feat1, dtype=np.float32)
    feat2 = np.asarray(feat2, dtype=np.float32)
    mask = np.asarray(mask)
    Wq = np.asarray(Wq, dtype=np.float32)
    bq = np.asarray(bq, dtype=np.float32)
    Wkv = np.asarray(Wkv, dtype=np.float32)
    bkv = np.asarray(bkv, dtype=np.float32)
    Weg = np.asarray(Weg, dtype=np.float32)
    beg = np.asarray(beg, dtype=np.float32)
    ln_gamma = np.asarray(ln_gamma, dtype=np.float32)
    ln_beta = np.asarray(ln_beta, dtype=np.float32)

    if not mask.all():
        return _numpy_fallback(feat1, feat2, mask, Wq, bq, Wkv, bkv, Weg, beg,
                               ln_gamma, ln_beta)

    from concourse import bass_utils

    if int(os.environ.get("KLDWOPT", "0")) and not getattr(bass_utils, "_ldwopt_patched", False):
        _orig_run_command = bass_utils.run_command

        def _run_command_ldwopt(argv, **kwargs):
            argv = ["--enable-ldw-opt=true" if a == "--enable-ldw-opt=false" else a
                    for a in argv]
            return _orig_run_command(argv, **kwargs)

        bass_utils.run_command = _run_command_ldwopt
        bass_utils._ldwopt_patched = True

    bf16 = ml_dtypes.bfloat16
    s = D ** -0.5
    # head-contiguous channel permutation: c' = h*32+d  <->  c = d*8+h
    cp = np.arange(C)
    perm = (cp % D) * H + (cp // D)          # perm[c'] = original channel

    Wq_s = (Wq * s)[perm, :]                 # rows reordered to c' order
    Wk_s = Wkv[0:C][perm, :]
    Wv_s = Wkv[C:2 * C][perm, :]
    wqt_np = np.ascontiguousarray(Wq_s.T).astype(bf16)
    wkt_np = np.ascontiguousarray(Wk_s.T).astype(bf16)
    wvt_np = np.ascontiguousarray(Wv_s.T).astype(bf16)
    wegt_np = np.ascontiguousarray(Weg.T).astype(bf16)

    has_bq = bool(np.any(bq))
    has_bk = bool(np.any(bkv[0:C]))
    has_bveg = bool(np.any(bkv[C:2 * C])) or bool(np.any(beg))
    has_gb = (not np.all(ln_gamma == 1.0)) or bool(np.any(ln_beta))
    flags = (has_bq, has_bk, has_bveg, has_gb)

    if flags not in _BUILD_CACHE:
        _BUILD_CACHE[flags] = _build(flags)
    nc = _BUILD_CACHE[flags]

    in_maps = []
    for m in range(NCORES):
        js = slice(m * JP, (m + 1) * JP)
        f1s = feat1[:, :, js, :]                       # [B, I, JP, C]
        f1t_np = np.ascontiguousarray(f1s.transpose(0, 3, 2, 1)).reshape(B, C, JPN).astype(bf16)
        f2s = feat2[:, js, :, :]                       # [B, JP, K, C]
        f2t_np = np.ascontiguousarray(f2s.transpose(0, 3, 1, 2)).reshape(B, C, JPN).astype(bf16)
        im = {"f1t": f1t_np, "f2t": f2t_np, "wqt": wqt_np, "wkt": wkt_np,
              "wvt": wvt_np, "wegt": wegt_np}
        if has_bq:
            im["bq_p"] = np.ascontiguousarray((bq * s)[perm])
        if has_bk:
            im["bk_p"] = np.ascontiguousarray(bkv[0:C][perm])
        if has_bveg:
            im["bveg_p"] = np.concatenate([bkv[C:2 * C][perm], beg]).astype(bf16)
        if has_gb:
            im["gamma_p"] = ln_gamma
            im["beta_p"] = ln_beta
        in_maps.append(im)

    trace = bool(int(os.environ.get("KBENCH_TRACE", "0")))
    res = bass_utils.run_bass_kernel_spmd(nc, in_maps, core_ids=list(range(NCORES)),
                                          trace=trace)
    if trace:
        kernel.last_exec_time_ns = res.exec_time_ns

    out = np.empty((B, N, N, C), dtype=np.float32)
    for m in range(NCORES):
        js = slice(m * JP, (m + 1) * JP)
        out[:, :, js, :] = res.results[m]["out"]
    return out
